# revision 8
# baseline (speedup 1.0000x reference)
"""Trainium2 Bass kernel for nn_LinkPredictor.

Reference computation (B=4, N=256, T=16, F=128, H=256):
    h = mean_T(nodefeat)                      # [B,N,F]
    a = h @ W1[:, :F].T                       # [B,N,H]
    c = h @ W1[:, F:].T                       # [B,N,H]
    logits[b,i,j] = W2[0] . relu(a[b,i] + c[b,j] + b1) + b2   # [B,N,N]

Sharding: 8 cores; core k handles batch b=k//2, i-half k%2 (128 i-rows x
256 j-cols of one batch's NxN grid).

Per-core plan (v3):
  - nf DMA: 6 chunks across 3 trigger queues (sync HW, gpsimd SW,
    scalar HW); w1/w2 packed into one bf16 tensor on the scalar queue.
  - hT via per-octet matmuls pipelined behind the DMA chunks, drained
    bf16 in halves; cT matmuls and ScalarE copies chunked j-128 so the
    first act op starts ~1us after the last nf byte.
  - aTb4 = (aT+b1) x4-replicated fp32 for the VE 16B-aligned scalar
    fast path.
  - Pairwise: act buffer [128, 1024] per pair-step = [ht0 i_a | ht0 i_b
    | ht1 i_a | ht1 i_b], filled by 4 ops split between VectorE
    (dual-op tensor_scalar ~194ns) and ScalarE (activation Relu+bias
    ~400ns) via greedy finish-time balancing.
  - Reduction: one matmul per (pair, ht): stationary [128,16] diag with
    w2_ht in column r', PSUM region [16, 512] per 32-i group; regions
    drain (ScalarE copy) + DMA out while later groups compute.
  - b2 and final assembly applied on host.
"""

import os
import sys

import numpy as np

_B, _N, _T, _F, _H = 4, 256, 16, 128, 256
_NCORES = 8

_VE_NS = 202.7  # measured per [128,256] dual tensor_scalar
_SE_NS = 411.5  # measured per [128,256] activation relu+bias
_SE_DRAIN_NS = 720.0
_VE_DRAIN_NS = 658.0

_CACHE = {}


def _ensure_paths():
    for p in (
        "/root/.axon_site",
        "/root/.axon_site/_ro/trn_rl_repo",
        "/root/.axon_site/_ro/pypackages",
        "/opt/trn_rl_repo",
    ):
        if os.path.isdir(p) and p not in sys.path:
            sys.path.append(p)


def build_nc():
    """Build the per-core Bass program (same program for all 8 cores)."""
    _ensure_paths()
    import concourse.mybir as mybir
    import concourse.tile as tile
    from concourse import bacc

    f32 = mybir.dt.float32
    bf16 = mybir.dt.bfloat16
    Alu = mybir.AluOpType
    Act = mybir.ActivationFunctionType

    nc = bacc.Bacc("TRN2", target_bir_lowering=False, debug=False)

    nf = nc.declare_dram_parameter("nf", [128, 32, 128], bf16, isOutput=False)
    smat = nc.declare_dram_parameter("smat", [128, 8], bf16, isOutput=False)
    # wpack[:, t, 0:128]=w1c_t^T, [:, t, 128:256]=w1a_t^T, [:, t, 256:512]=w2 diag
    wpack = nc.declare_dram_parameter("wpack", [128, 2, 512], bf16, isOutput=False)
    b1t = nc.declare_dram_parameter("b1t", [128, 2], f32, isOutput=False)
    outd = nc.declare_dram_parameter("out", [4, 16, 512], f32, isOutput=True)

    with tile.TileContext(nc) as tc:
        with (
            tc.tile_pool(name="const", bufs=1) as constp,
            tc.tile_pool(name="data", bufs=1) as datap,
            tc.tile_pool(name="act", bufs=12) as actp,
            tc.tile_pool(name="dr", bufs=2) as drp,
            tc.tile_pool(name="ph", bufs=1, space="PSUM") as php,
            tc.tile_pool(name="pc", bufs=2, space="PSUM") as pcp,
            tc.tile_pool(name="pl", bufs=2, space="PSUM") as plp,
        ):
            smat_sb = constp.tile([128, 8], bf16, tag="smat")
            nc.sync.dma_start(out=smat_sb[:], in_=smat[:])

            nf_sb = constp.tile([128, 32, 128], bf16, tag="nf")
            # 6 nf chunks, byte-balanced across the 3 trigger queues.
            chunks = [(0, 6), (6, 11), (11, 17), (17, 22), (22, 27), (27, 32)]
            engs = [nc.sync, nc.sync, nc.gpsimd, nc.gpsimd, nc.scalar, nc.scalar]
            for (o0, o1), eng in zip(chunks, engs):
                eng.dma_start(out=nf_sb[:, o0:o1, :], in_=nf[:, o0:o1, :])
            wpack_sb = constp.tile([128, 2, 512], bf16, tag="wpack")
            nc.scalar.dma_start(out=wpack_sb[:], in_=wpack[:])
            b1t_sb = constp.tile([128, 2], f32, tag="b1t")
            nc.scalar.dma_start(out=b1t_sb[:], in_=b1t[:])

            ph = php.tile([128, 256], f32, tag="ph")
            for o in range(32):
                nc.tensor.matmul(
                    ph[:, 8 * o : 8 * o + 8],
                    lhsT=nf_sb[:, o, :],
                    rhs=smat_sb[:],
                    start=True,
                    stop=True,
                )

            hT = datap.tile([128, 256], bf16, tag="hT")
            cT = [datap.tile([128, 256], bf16, tag=f"cT{t}", name=f"cT{t}") for t in range(2)]
            aTb4 = [datap.tile([128, 128, 4], f32, tag=f"aTb4{t}", name=f"aTb4{t}") for t in range(2)]
            pcs = [pcp.tile([128, 256], f32, tag="pc", name=f"pc{t}") for t in range(2)]

            # first j-half: hT drain, cT chunk mms, aT chain, cT copies
            nc.vector.tensor_copy(hT[:, 0:128], ph[:, 0:128])
            for t in range(2):
                nc.tensor.matmul(
                    pcs[t][:, 0:128], lhsT=wpack_sb[:, t, 0:128],
                    rhs=hT[:, 0:128], start=True, stop=True,
                )
            pa = [pcp.tile([128, 128], f32, tag="pa", name=f"pa{t}") for t in range(2)]
            for t in range(2):
                nc.tensor.matmul(
                    pa[t][:], lhsT=wpack_sb[:, t, 128:256],
                    rhs=hT[:, 0:128], start=True, stop=True,
                )
            nc.scalar.copy(cT[0][:, 0:128], pcs[0][:, 0:128])
            nc.scalar.copy(cT[1][:, 0:128], pcs[1][:, 0:128])
            for t in range(2):
                nc.vector.tensor_scalar(
                    aTb4[t][:, :, :],
                    pa[t][:].broadcast_to([128, 128, 4]),
                    b1t_sb[:, t : t + 1],
                    None,
                    Alu.add,
                )

            # second j-half
            nc.vector.tensor_copy(hT[:, 128:256], ph[:, 128:256])
            for t in range(2):
                nc.tensor.matmul(
                    pcs[t][:, 128:256], lhsT=wpack_sb[:, t, 0:128],
                    rhs=hT[:, 128:256], start=True, stop=True,
                )
            nc.scalar.copy(cT[0][:, 128:256], pcs[0][:, 128:256])
            nc.scalar.copy(cT[1][:, 128:256], pcs[1][:, 128:256])

            # Pairwise main loop: 4 groups x 16 pairs x 2 h-tiles.
            tV = 0.0
            tS = 1600.0  # SE busy with cT copies at loop start
            for g in range(4):
                pl = plp.tile([16, 512], f32, tag="pl", name=f"pl{g}")
                for rp in range(16):
                    buf = actp.tile([128, 1024], bf16, tag="act")
                    for t in range(2):
                        for s in range(2):
                            i = 32 * g + 2 * rp + s
                            a_col = aTb4[t][:, i, 0:1]
                            dst = buf[:, 512 * t + 256 * s : 512 * t + 256 * s + 256]
                            if tV + _VE_NS <= tS + _SE_NS:
                                tV += _VE_NS
                                nc.vector.tensor_scalar(
                                    dst, cT[t][:], a_col, 0.0, Alu.add, Alu.max
                                )
                            else:
                                tS += _SE_NS
                                nc.scalar.activation(dst, cT[t][:], Act.Relu, bias=a_col)
                        nc.tensor.matmul(
                            pl[:, :],
                            lhsT=wpack_sb[:, t, 256 + 16 * rp : 272 + 16 * rp],
                            rhs=buf[:, 512 * t : 512 * t + 512],
                            start=(rp == 0 and t == 0),
                            stop=(rp == 15 and t == 1),
                        )
                osb = drp.tile([16, 512], f32, tag="osb", name=f"osb{g}")
                if g < 3:
                    nc.scalar.copy(osb[:], pl[:])
                    tS += _SE_DRAIN_NS
                else:
                    # final drain on VectorE: it retires its act stream first,
                    # shortening the drain->DMA->barrier tail
                    nc.vector.tensor_copy(osb[:], pl[:])
                    tV += _VE_DRAIN_NS
                nc.sync.dma_start(out=outd[g], in_=osb[:])

    nc.compile()
    return nc


def make_in_maps(nodefeat, W1, b1, W2, b2):
    """Host-side sharding/layout prep (layout + dtype only)."""
    import ml_dtypes

    bf16 = ml_dtypes.bfloat16
    nodefeat = np.asarray(nodefeat, dtype=np.float32)
    W1 = np.asarray(W1, dtype=np.float32)
    b1 = np.asarray(b1, dtype=np.float32)
    W2 = np.asarray(W2, dtype=np.float32)

    smat = (np.repeat(np.eye(8, dtype=np.float32), 16, axis=0) / 16.0).astype(bf16)

    W1a, W1c = W1[:, :_F], W1[:, _F:]
    w1at = np.stack([W1a[:128].T, W1a[128:].T], axis=1)  # [128 f, 2, 128 h]
    w1ct = np.stack([W1c[:128].T, W1c[128:].T], axis=1)
    b1t = np.ascontiguousarray(b1.reshape(2, 128).T)

    w2r = W2[0].reshape(2, 128)  # [ht, p]
    w2b = np.zeros((128, 2, 16, 16), dtype=np.float32)
    idx = np.arange(16)
    w2b[:, :, idx, idx] = w2r.T[:, :, None]

    wpack = np.concatenate(
        [w1ct, w1at, w2b.reshape(128, 2, 256)], axis=2
    ).astype(bf16)  # [128, 2, 512]

    in_maps = []
    for k in range(_NCORES):
        b, ih = divmod(k, 2)
        nf_b = nodefeat[b]  # [256, 16, 128]
        if ih:
            nf_b = np.concatenate([nf_b[128:], nf_b[:128]], axis=0)
        # [256,16,128] -> [32 oct, (j8,t16)=128, 128 f] -> [128, 32, 128]
        nf_dev = np.ascontiguousarray(
            nf_b.reshape(32, 128, 128).transpose(1, 0, 2).astype(bf16)
        )
        in_maps.append(
            {
                "nf": nf_dev,
                "smat": smat,
                "wpack": wpack,
                "b1t": b1t,
            }
        )
    return in_maps


def core_output_to_ij(arr, b2_val):
    """Device output [4, 16, 512] -> core-local logits [128 i, 256 j]."""
    return arr.reshape(128, 256).astype(np.float32) + b2_val


def assemble_output(results, b2):
    b2_val = float(np.asarray(b2).reshape(-1)[0])
    out = np.empty((_B, _N, _N), dtype=np.float32)
    for k in range(_NCORES):
        b, ih = divmod(k, 2)
        r = core_output_to_ij(results[k]["out"], b2_val)  # [i, j] core-local j order
        if ih:
            r = np.concatenate([r[:, 128:], r[:, :128]], axis=1)
        out[b, ih * 128 : (ih + 1) * 128, :] = r
    return out


def _get_nc():
    if "nc" not in _CACHE:
        _CACHE["nc"] = build_nc()
    return _CACHE["nc"]


def kernel(nodefeat, W1, b1, W2, b2):
    _ensure_paths()
    from concourse.bass_utils import run_bass_kernel_spmd

    nc = _get_nc()
    in_maps = make_in_maps(nodefeat, W1, b1, W2, b2)
    res = run_bass_kernel_spmd(nc, in_maps, list(range(_NCORES)))
    return assemble_output(res.results, b2)


# revision 11
# speedup vs baseline: 1.0145x; 1.0145x over previous
"""Trainium2 Bass kernel for nn_LinkPredictor.

Reference computation (B=4, N=256, T=16, F=128, H=256):
    h = mean_T(nodefeat)                      # [B,N,F]
    a = h @ W1[:, :F].T                       # [B,N,H]
    c = h @ W1[:, F:].T                       # [B,N,H]
    logits[b,i,j] = W2[0] . relu(a[b,i] + c[b,j] + b1) + b2   # [B,N,N]

Sharding: 8 cores; core k handles batch b=k//2, i-half k%2 (128 i-rows x
256 j-cols of one batch's NxN grid).

Per-core plan (v5):
  - nf and the mean-selection matrix in fp8-e4m3 (error budget checked in
    sim; DMA bytes halve vs bf16). 5 nf chunks + weights balanced across
    3 trigger queues (sync HW, gpsimd SW, scalar HW) at ~256KB each.
  - hT via per-octet matmuls (fp8 stationary x fp8 moving) pipelined
    behind the DMA chunks, drained bf16 in j-128 halves; cT matmuls and
    ScalarE copies chunked so the first act op trails the last nf byte
    by ~1us.
  - aTb4 = (aT+b1) x4-replicated fp32 for the VE 16B-aligned scalar
    fast path.
  - Pairwise: act buffer [128, 1024] per pair-step = [ht0 i_a | ht0 i_b
    | ht1 i_a | ht1 i_b], filled by 4 ops split between VectorE
    (dual-op tensor_scalar ~203ns) and ScalarE (activation Relu+bias
    ~412ns) via greedy finish-time balancing.
  - Reduction: one matmul per (pair, ht): stationary [128,n_g] diag with
    w2_ht in column r', PSUM region [n_g, 512] per group; group sizes
    16/16/16/12/4 so drains overlap compute and the final
    drain+DMA tail is small.
  - b2 and final assembly applied on host.
"""

import os
import sys

import numpy as np

_B, _N, _T, _F, _H = 4, 256, 16, 128, 256
_NCORES = 8

_VE_NS = 202.7  # measured per [128,256] dual tensor_scalar
_SE_NS = 411.5  # measured per [128,256] activation relu+bias
_SE_DRAIN_NS = 720.0
_VE_DRAIN_NS = 658.0

_GROUPS = [16, 16, 16, 12, 4]  # pairs per PSUM region

_CACHE = {}


def _ensure_paths():
    for p in (
        "/root/.axon_site",
        "/root/.axon_site/_ro/trn_rl_repo",
        "/root/.axon_site/_ro/pypackages",
        "/opt/trn_rl_repo",
    ):
        if os.path.isdir(p) and p not in sys.path:
            sys.path.append(p)


def build_nc():
    """Build the per-core Bass program (same program for all 8 cores)."""
    _ensure_paths()
    import concourse.mybir as mybir
    import concourse.tile as tile
    from concourse import bacc

    f32 = mybir.dt.float32
    bf16 = mybir.dt.bfloat16
    fp8 = mybir.dt.float8e4
    Alu = mybir.AluOpType
    Act = mybir.ActivationFunctionType

    nc = bacc.Bacc("TRN2", target_bir_lowering=False, debug=False)

    nf = nc.declare_dram_parameter("nf", [128, 32, 128], fp8, isOutput=False)
    smat = nc.declare_dram_parameter("smat", [128, 8], fp8, isOutput=False)
    # wpack[:, t, 0:128]=w1c_t^T, [:, t, 128:256]=w1a_t^T, [:, t, 256:512]=w2 diag
    wpack = nc.declare_dram_parameter("wpack", [128, 2, 512], bf16, isOutput=False)
    b1t = nc.declare_dram_parameter("b1t", [128, 2], f32, isOutput=False)
    outd = nc.declare_dram_parameter("out", [64, 512], f32, isOutput=True)

    with tile.TileContext(nc) as tc:
        with (
            tc.tile_pool(name="const", bufs=1) as constp,
            tc.tile_pool(name="data", bufs=1) as datap,
            tc.tile_pool(name="act", bufs=12) as actp,
            tc.tile_pool(name="dr", bufs=2) as drp,
            tc.tile_pool(name="ph", bufs=1, space="PSUM") as php,
            tc.tile_pool(name="pc", bufs=2, space="PSUM") as pcp,
            tc.tile_pool(name="pl", bufs=2, space="PSUM") as plp,
        ):
            smat_sb = constp.tile([128, 8], fp8, tag="smat")
            nc.sync.dma_start(out=smat_sb[:], in_=smat[:])

            nf_sb = constp.tile([128, 32, 128], fp8, tag="nf")
            wpack_sb = constp.tile([128, 2, 512], bf16, tag="wpack")
            b1t_sb = constp.tile([128, 2], f32, tag="b1t")
            # ~256KB per queue: sync oct 0-15, gpsimd oct 16-31, scalar weights.
            nc.sync.dma_start(out=nf_sb[:, 0:8, :], in_=nf[:, 0:8, :])
            nc.gpsimd.dma_start(out=nf_sb[:, 16:24, :], in_=nf[:, 16:24, :])
            nc.scalar.dma_start(out=wpack_sb[:, :, 0:256], in_=wpack[:, :, 0:256])
            nc.sync.dma_start(out=nf_sb[:, 8:16, :], in_=nf[:, 8:16, :])
            nc.gpsimd.dma_start(out=nf_sb[:, 24:32, :], in_=nf[:, 24:32, :])
            nc.scalar.dma_start(out=wpack_sb[:, :, 256:512], in_=wpack[:, :, 256:512])
            nc.scalar.dma_start(out=b1t_sb[:], in_=b1t[:])

            ph = php.tile([128, 256], f32, tag="ph")
            for o in range(32):
                nc.tensor.matmul(
                    ph[:, 8 * o : 8 * o + 8],
                    lhsT=nf_sb[:, o, :],
                    rhs=smat_sb[:],
                    start=True,
                    stop=True,
                )

            hT = datap.tile([128, 256], bf16, tag="hT")
            cT = [datap.tile([128, 256], bf16, tag=f"cT{t}", name=f"cT{t}") for t in range(2)]
            aTb4 = [datap.tile([128, 128, 4], f32, tag=f"aTb4{t}", name=f"aTb4{t}") for t in range(2)]
            pcs = [pcp.tile([128, 256], f32, tag="pc", name=f"pc{t}") for t in range(2)]

            # first j-half: hT drain, cT chunk mms, aT chain, cT copies
            nc.vector.tensor_copy(hT[:, 0:128], ph[:, 0:128])
            for t in range(2):
                nc.tensor.matmul(
                    pcs[t][:, 0:128], lhsT=wpack_sb[:, t, 0:128],
                    rhs=hT[:, 0:128], start=True, stop=True,
                )
            pa = [pcp.tile([128, 128], f32, tag="pa", name=f"pa{t}") for t in range(2)]
            for t in range(2):
                nc.tensor.matmul(
                    pa[t][:], lhsT=wpack_sb[:, t, 128:256],
                    rhs=hT[:, 0:128], start=True, stop=True,
                )
            nc.scalar.copy(cT[0][:, 0:128], pcs[0][:, 0:128])
            nc.scalar.copy(cT[1][:, 0:128], pcs[1][:, 0:128])
            for t in range(2):
                nc.vector.tensor_scalar(
                    aTb4[t][:, :, :],
                    pa[t][:].broadcast_to([128, 128, 4]),
                    b1t_sb[:, t : t + 1],
                    None,
                    Alu.add,
                )

            # second j-half
            nc.vector.tensor_copy(hT[:, 128:256], ph[:, 128:256])
            for t in range(2):
                nc.tensor.matmul(
                    pcs[t][:, 128:256], lhsT=wpack_sb[:, t, 0:128],
                    rhs=hT[:, 128:256], start=True, stop=True,
                )
            nc.scalar.copy(cT[0][:, 128:256], pcs[0][:, 128:256])
            nc.scalar.copy(cT[1][:, 128:256], pcs[1][:, 128:256])

            # Pairwise main loop over groups of pairs.
            tV = 0.0
            tS = 800.0  # SE busy with second-half cT copies at loop start
            p0 = 0
            for g, ng in enumerate(_GROUPS):
                last_group = g == len(_GROUPS) - 1
                pl = plp.tile([ng, 512], f32, tag="pl", name=f"pl{g}")
                for rp in range(ng):
                    buf = actp.tile([128, 1024], bf16, tag="act")
                    for t in range(2):
                        for s in range(2):
                            i = 2 * (p0 + rp) + s
                            a_col = aTb4[t][:, i, 0:1]
                            dst = buf[:, 512 * t + 256 * s : 512 * t + 256 * s + 256]
                            if tV + _VE_NS <= tS + _SE_NS:
                                tV += _VE_NS
                                nc.vector.tensor_scalar(
                                    dst, cT[t][:], a_col, 0.0, Alu.add, Alu.max
                                )
                            else:
                                tS += _SE_NS
                                nc.scalar.activation(dst, cT[t][:], Act.Relu, bias=a_col)
                        nc.tensor.matmul(
                            pl[:, :],
                            lhsT=wpack_sb[:, t, 256 + 16 * rp : 256 + 16 * rp + ng],
                            rhs=buf[:, 512 * t : 512 * t + 512],
                            start=(rp == 0 and t == 0),
                            stop=(rp == ng - 1 and t == 1),
                        )
                osb = drp.tile([ng, 512], f32, tag="osb", name=f"osb{g}")
                if last_group or tV + _VE_DRAIN_NS <= tS + _SE_DRAIN_NS:
                    nc.vector.tensor_copy(osb[:], pl[:])
                    tV += _VE_DRAIN_NS
                else:
                    nc.scalar.copy(osb[:], pl[:])
                    tS += _SE_DRAIN_NS
                nc.sync.dma_start(out=outd[p0 : p0 + ng], in_=osb[:])
                p0 += ng

    nc.compile()
    return nc


def make_in_maps(nodefeat, W1, b1, W2, b2):
    """Host-side sharding/layout prep (layout + dtype only)."""
    import ml_dtypes

    bf16 = ml_dtypes.bfloat16
    fp8 = ml_dtypes.float8_e4m3fn
    nodefeat = np.asarray(nodefeat, dtype=np.float32)
    W1 = np.asarray(W1, dtype=np.float32)
    b1 = np.asarray(b1, dtype=np.float32)
    W2 = np.asarray(W2, dtype=np.float32)

    smat = (np.repeat(np.eye(8, dtype=np.float32), 16, axis=0) / 16.0).astype(fp8)

    W1a, W1c = W1[:, :_F], W1[:, _F:]
    w1at = np.stack([W1a[:128].T, W1a[128:].T], axis=1)  # [128 f, 2, 128 h]
    w1ct = np.stack([W1c[:128].T, W1c[128:].T], axis=1)
    b1t = np.ascontiguousarray(b1.reshape(2, 128).T)

    w2r = W2[0].reshape(2, 128)  # [ht, p]
    w2b = np.zeros((128, 2, 16, 16), dtype=np.float32)
    idx = np.arange(16)
    w2b[:, :, idx, idx] = w2r.T[:, :, None]

    wpack = np.concatenate(
        [w1ct, w1at, w2b.reshape(128, 2, 256)], axis=2
    ).astype(bf16)  # [128, 2, 512]

    # fp8 with error feedback along T: each slice is individually fp8-close
    # to its true value, and the T-sum the device computes stays accurate.
    nfq = np.empty_like(nodefeat)
    carry = np.zeros(nodefeat[:, :, 0, :].shape, dtype=np.float32)
    for t in range(_T):
        x = nodefeat[:, :, t, :] + carry
        qx = x.astype(fp8).astype(np.float32)
        carry = x - qx
        nfq[:, :, t, :] = qx

    in_maps = []
    for k in range(_NCORES):
        b, ih = divmod(k, 2)
        nf_b = nfq[b]  # [256, 16, 128]
        if ih:
            nf_b = np.concatenate([nf_b[128:], nf_b[:128]], axis=0)
        # [256,16,128] -> [32 oct, (j8,t16)=128, 128 f] -> [128, 32, 128]
        nf_dev = np.ascontiguousarray(
            nf_b.reshape(32, 128, 128).transpose(1, 0, 2).astype(fp8)
        )
        in_maps.append(
            {
                "nf": nf_dev,
                "smat": smat,
                "wpack": wpack,
                "b1t": b1t,
            }
        )
    return in_maps


def core_output_to_ij(arr, b2_val):
    """Device output [64, 512] -> core-local logits [128 i, 256 j]."""
    return arr.reshape(128, 256).astype(np.float32) + b2_val


def assemble_output(results, b2):
    b2_val = float(np.asarray(b2).reshape(-1)[0])
    out = np.empty((_B, _N, _N), dtype=np.float32)
    for k in range(_NCORES):
        b, ih = divmod(k, 2)
        r = core_output_to_ij(results[k]["out"], b2_val)  # [i, j] core-local j order
        if ih:
            r = np.concatenate([r[:, 128:], r[:, :128]], axis=1)
        out[b, ih * 128 : (ih + 1) * 128, :] = r
    return out


def _get_nc():
    if "nc" not in _CACHE:
        _CACHE["nc"] = build_nc()
    return _CACHE["nc"]


def kernel(nodefeat, W1, b1, W2, b2):
    _ensure_paths()
    from concourse.bass_utils import run_bass_kernel_spmd

    nc = _get_nc()
    in_maps = make_in_maps(nodefeat, W1, b1, W2, b2)
    res = run_bass_kernel_spmd(nc, in_maps, list(range(_NCORES)))
    return assemble_output(res.results, b2)


# revision 14
# speedup vs baseline: 1.0196x; 1.0049x over previous
"""Trainium2 Bass kernel for nn_LinkPredictor.

Reference computation (B=4, N=256, T=16, F=128, H=256):
    h = mean_T(nodefeat)                      # [B,N,F]
    a = h @ W1[:, :F].T                       # [B,N,H]
    c = h @ W1[:, F:].T                       # [B,N,H]
    logits[b,i,j] = W2[0] . relu(a[b,i] + c[b,j] + b1) + b2   # [B,N,N]

Sharding: 8 cores; core k handles batch b=k//2, i-half k%2 (128 i-rows x
256 j-cols of one batch's NxN grid).

Per-core plan (v5):
  - nf and the mean-selection matrix in fp8-e4m3 (error budget checked in
    sim; DMA bytes halve vs bf16). 5 nf chunks + weights balanced across
    3 trigger queues (sync HW, gpsimd SW, scalar HW) at ~256KB each.
  - hT via per-octet matmuls (fp8 stationary x fp8 moving) pipelined
    behind the DMA chunks, drained bf16 in j-128 halves; cT matmuls and
    ScalarE copies chunked so the first act op trails the last nf byte
    by ~1us.
  - aTb4 = (aT+b1) x4-replicated fp32 for the VE 16B-aligned scalar
    fast path.
  - Pairwise: act buffer [128, 1024] per pair-step = [ht0 i_a | ht0 i_b
    | ht1 i_a | ht1 i_b], filled by 4 ops split between VectorE
    (dual-op tensor_scalar ~203ns) and ScalarE (activation Relu+bias
    ~412ns) via greedy finish-time balancing.
  - Reduction: one matmul per (pair, ht): stationary [128,n_g] diag with
    w2_ht in column r', PSUM region [n_g, 512] per group; group sizes
    16/16/16/12/4 so drains overlap compute and the final
    drain+DMA tail is small.
  - b2 and final assembly applied on host.
"""

import os
import sys

import numpy as np

_B, _N, _T, _F, _H = 4, 256, 16, 128, 256
_NCORES = 8

_VE_NS = 202.7  # measured per [128,256] dual tensor_scalar
_SE_NS = 411.5  # measured per [128,256] activation relu+bias
_SE_DRAIN_NS = 720.0
_VE_DRAIN_NS = 658.0

_GROUPS = [16, 16, 16, 12, 4]  # pairs per PSUM region

_CACHE = {}


def _ensure_paths():
    for p in (
        "/root/.axon_site",
        "/root/.axon_site/_ro/trn_rl_repo",
        "/root/.axon_site/_ro/pypackages",
        "/opt/trn_rl_repo",
    ):
        if os.path.isdir(p) and p not in sys.path:
            sys.path.append(p)


def build_nc():
    """Build the per-core Bass program (same program for all 8 cores)."""
    _ensure_paths()
    import concourse.mybir as mybir
    import concourse.tile as tile
    from concourse import bacc

    f32 = mybir.dt.float32
    bf16 = mybir.dt.bfloat16
    fp8 = mybir.dt.float8e4
    Alu = mybir.AluOpType
    Act = mybir.ActivationFunctionType

    nc = bacc.Bacc("TRN2", target_bir_lowering=False, debug=False)

    nf = nc.declare_dram_parameter("nf", [128, 32, 128], fp8, isOutput=False)
    smat = nc.declare_dram_parameter("smat", [128, 8], fp8, isOutput=False)
    # wpack[:, t, 0:128]=w1c_t^T, [:, t, 128:256]=w1a_t^T, [:, t, 256:512]=w2 diag
    wpack = nc.declare_dram_parameter("wpack", [128, 2, 512], bf16, isOutput=False)
    b1t = nc.declare_dram_parameter("b1t", [128, 2], f32, isOutput=False)
    outd = nc.declare_dram_parameter("out", [64, 512], f32, isOutput=True)

    with tile.TileContext(nc) as tc:
        with (
            tc.tile_pool(name="const", bufs=1) as constp,
            tc.tile_pool(name="data", bufs=1) as datap,
            tc.tile_pool(name="act", bufs=12) as actp,
            tc.tile_pool(name="dr", bufs=2) as drp,
            tc.tile_pool(name="ph", bufs=1, space="PSUM") as php,
            tc.tile_pool(name="pc", bufs=2, space="PSUM") as pcp,
            tc.tile_pool(name="pl", bufs=2, space="PSUM") as plp,
        ):
            smat_sb = constp.tile([128, 8], fp8, tag="smat")
            nc.sync.dma_start(out=smat_sb[:], in_=smat[:])

            nf_sb = constp.tile([128, 32, 128], fp8, tag="nf")
            wpack_sb = constp.tile([128, 2, 512], bf16, tag="wpack")
            b1t_sb = constp.tile([128, 2], f32, tag="b1t")
            # Queue plan (per-queue ~1.6us latency + ~85GB/s): b1t + the w1
            # half of wpack lead the scalar queue (needed first); nf split
            # ~byte-balanced across all three queues; w2 diag half arrives
            # before the first pairwise matmul.
            nc.scalar.dma_start(out=b1t_sb[:], in_=b1t[:])
            nc.scalar.dma_start(out=wpack_sb[:, :, 0:256], in_=wpack[:, :, 0:256])
            nc.sync.dma_start(out=nf_sb[:, 0:6, :], in_=nf[:, 0:6, :])
            nc.gpsimd.dma_start(out=nf_sb[:, 11:17, :], in_=nf[:, 11:17, :])
            nc.sync.dma_start(out=nf_sb[:, 6:11, :], in_=nf[:, 6:11, :])
            nc.gpsimd.dma_start(out=nf_sb[:, 17:22, :], in_=nf[:, 17:22, :])
            nc.scalar.dma_start(out=nf_sb[:, 22:32, :], in_=nf[:, 22:32, :])
            nc.scalar.dma_start(out=wpack_sb[:, :, 256:512], in_=wpack[:, :, 256:512])

            # split PSUM tiles so j-half consumers wait only their own half;
            # octet order follows expected DMA chunk arrival
            phA = php.tile([128, 128], f32, tag="phA")
            phB = php.tile([128, 128], f32, tag="phB")
            oct_order = (
                list(range(0, 6)) + list(range(11, 16)) + list(range(6, 11))
                + [16] + list(range(17, 22)) + list(range(22, 32))
            )
            for o in oct_order:
                ph = phA if o < 16 else phB
                nc.tensor.matmul(
                    ph[:, 8 * (o % 16) : 8 * (o % 16) + 8],
                    lhsT=nf_sb[:, o, :],
                    rhs=smat_sb[:],
                    start=True,
                    stop=True,
                )

            hT = datap.tile([128, 256], bf16, tag="hT")
            cT = [datap.tile([128, 256], bf16, tag=f"cT{t}", name=f"cT{t}") for t in range(2)]
            aTb4 = [datap.tile([128, 128, 4], f32, tag=f"aTb4{t}", name=f"aTb4{t}") for t in range(2)]
            # separate [128,128] PSUM tiles per (t, j-half): no false WAR deps
            pcs = [
                [pcp.tile([128, 128], f32, tag="pc", name=f"pc{t}h{h}") for h in range(2)]
                for t in range(2)
            ]

            # first j-half: hT drain, cT chunk mms, aT chain, cT copies
            nc.vector.tensor_copy(hT[:, 0:128], phA[:])
            for t in range(2):
                nc.tensor.matmul(
                    pcs[t][0][:], lhsT=wpack_sb[:, t, 0:128],
                    rhs=hT[:, 0:128], start=True, stop=True,
                )
            pa = [pcp.tile([128, 128], f32, tag="pa", name=f"pa{t}") for t in range(2)]
            for t in range(2):
                nc.tensor.matmul(
                    pa[t][:], lhsT=wpack_sb[:, t, 128:256],
                    rhs=hT[:, 0:128], start=True, stop=True,
                )
            nc.scalar.copy(cT[0][:, 0:128], pcs[0][0][:])
            nc.scalar.copy(cT[1][:, 0:128], pcs[1][0][:])
            for t in range(2):
                nc.vector.tensor_scalar(
                    aTb4[t][:, :, :],
                    pa[t][:].broadcast_to([128, 128, 4]),
                    b1t_sb[:, t : t + 1],
                    None,
                    Alu.add,
                )

            # second j-half
            nc.vector.tensor_copy(hT[:, 128:256], phB[:])
            for t in range(2):
                nc.tensor.matmul(
                    pcs[t][1][:], lhsT=wpack_sb[:, t, 0:128],
                    rhs=hT[:, 128:256], start=True, stop=True,
                )
            nc.scalar.copy(cT[0][:, 128:256], pcs[0][1][:])
            nc.scalar.copy(cT[1][:, 128:256], pcs[1][1][:])

            # Pairwise main loop over groups of pairs.
            tV = 0.0
            tS = 600.0  # SE busy with second-half cT copies at loop start
            p0 = 0
            for g, ng in enumerate(_GROUPS):
                last_group = g == len(_GROUPS) - 1
                pl = plp.tile([ng, 512], f32, tag="pl", name=f"pl{g}")
                for rp in range(ng):
                    buf = actp.tile([128, 1024], bf16, tag="act")
                    for t in range(2):
                        for s in range(2):
                            i = 2 * (p0 + rp) + s
                            a_col = aTb4[t][:, i, 0:1]
                            dst = buf[:, 512 * t + 256 * s : 512 * t + 256 * s + 256]
                            if tV + _VE_NS <= tS + _SE_NS:
                                tV += _VE_NS
                                nc.vector.tensor_scalar(
                                    dst, cT[t][:], a_col, 0.0, Alu.add, Alu.max
                                )
                            else:
                                tS += _SE_NS
                                nc.scalar.activation(dst, cT[t][:], Act.Relu, bias=a_col)
                        nc.tensor.matmul(
                            pl[:, :],
                            lhsT=wpack_sb[:, t, 256 + 16 * rp : 256 + 16 * rp + ng],
                            rhs=buf[:, 512 * t : 512 * t + 512],
                            start=(rp == 0 and t == 0),
                            stop=(rp == ng - 1 and t == 1),
                        )
                osb = drp.tile([ng, 512], f32, tag="osb", name=f"osb{g}")
                if last_group or tV + _VE_DRAIN_NS <= tS + _SE_DRAIN_NS:
                    nc.vector.tensor_copy(osb[:], pl[:])
                    tV += _VE_DRAIN_NS
                else:
                    nc.scalar.copy(osb[:], pl[:])
                    tS += _SE_DRAIN_NS
                nc.sync.dma_start(out=outd[p0 : p0 + ng], in_=osb[:])
                p0 += ng

    nc.compile()
    return nc


def make_in_maps(nodefeat, W1, b1, W2, b2):
    """Host-side sharding/layout prep (layout + dtype only)."""
    import ml_dtypes

    bf16 = ml_dtypes.bfloat16
    fp8 = ml_dtypes.float8_e4m3fn
    nodefeat = np.asarray(nodefeat, dtype=np.float32)
    W1 = np.asarray(W1, dtype=np.float32)
    b1 = np.asarray(b1, dtype=np.float32)
    W2 = np.asarray(W2, dtype=np.float32)

    smat = (np.repeat(np.eye(8, dtype=np.float32), 16, axis=0) / 16.0).astype(fp8)

    W1a, W1c = W1[:, :_F], W1[:, _F:]
    w1at = np.stack([W1a[:128].T, W1a[128:].T], axis=1)  # [128 f, 2, 128 h]
    w1ct = np.stack([W1c[:128].T, W1c[128:].T], axis=1)
    b1t = np.ascontiguousarray(b1.reshape(2, 128).T)

    w2r = W2[0].reshape(2, 128)  # [ht, p]
    w2b = np.zeros((128, 2, 16, 16), dtype=np.float32)
    idx = np.arange(16)
    w2b[:, :, idx, idx] = w2r.T[:, :, None]

    wpack = np.concatenate(
        [w1ct, w1at, w2b.reshape(128, 2, 256)], axis=2
    ).astype(bf16)  # [128, 2, 512]

    # fp8 with error feedback along T: each slice is individually fp8-close
    # to its true value, and the T-sum the device computes stays accurate.
    nfq = np.empty_like(nodefeat)
    carry = np.zeros(nodefeat[:, :, 0, :].shape, dtype=np.float32)
    for t in range(_T):
        x = nodefeat[:, :, t, :] + carry
        qx = x.astype(fp8).astype(np.float32)
        carry = x - qx
        nfq[:, :, t, :] = qx

    in_maps = []
    for k in range(_NCORES):
        b, ih = divmod(k, 2)
        nf_b = nfq[b]  # [256, 16, 128]
        if ih:
            nf_b = np.concatenate([nf_b[128:], nf_b[:128]], axis=0)
        # [256,16,128] -> [32 oct, (j8,t16)=128, 128 f] -> [128, 32, 128]
        nf_dev = np.ascontiguousarray(
            nf_b.reshape(32, 128, 128).transpose(1, 0, 2).astype(fp8)
        )
        in_maps.append(
            {
                "nf": nf_dev,
                "smat": smat,
                "wpack": wpack,
                "b1t": b1t,
            }
        )
    return in_maps


def core_output_to_ij(arr, b2_val):
    """Device output [64, 512] -> core-local logits [128 i, 256 j]."""
    return arr.reshape(128, 256).astype(np.float32) + b2_val


def assemble_output(results, b2):
    b2_val = float(np.asarray(b2).reshape(-1)[0])
    out = np.empty((_B, _N, _N), dtype=np.float32)
    for k in range(_NCORES):
        b, ih = divmod(k, 2)
        r = core_output_to_ij(results[k]["out"], b2_val)  # [i, j] core-local j order
        if ih:
            r = np.concatenate([r[:, 128:], r[:, :128]], axis=1)
        out[b, ih * 128 : (ih + 1) * 128, :] = r
    return out


def _get_nc():
    if "nc" not in _CACHE:
        _CACHE["nc"] = build_nc()
    return _CACHE["nc"]


def kernel(nodefeat, W1, b1, W2, b2):
    _ensure_paths()
    from concourse.bass_utils import run_bass_kernel_spmd

    nc = _get_nc()
    in_maps = make_in_maps(nodefeat, W1, b1, W2, b2)
    res = run_bass_kernel_spmd(nc, in_maps, list(range(_NCORES)))
    return assemble_output(res.results, b2)
